# revision 1
# baseline (speedup 1.0000x reference)
"""v2 fallback: bf16 kernel, simple ft-major loop. Measured 136347 ns."""

import numpy as np
import ml_dtypes

import concourse.bacc as bacc
import concourse.mybir as mybir
import concourse.tile as tile
from concourse.bass_utils import run_bass_kernel_spmd

B, T, NX, NF, KC = 8, 1024, 1024, 4096, 50
N_CORES = 8
P = 128
KT = NX // P
FT = NF // P
TCH = 512
NTC = T // TCH

F32 = mybir.dt.float32
F32R = mybir.dt.float32r
BF16 = mybir.dt.bfloat16

TRACE = False
LAST_RESULT = None

_cached = None


def _build():
    nc = bacc.Bacc("TRN2", target_bir_lowering=False, debug=False,
                   num_devices=N_CORES)

    xh = nc.dram_tensor("xh", [P, KT, T], BF16, kind="ExternalInput").ap()
    wt = nc.dram_tensor("wt", [FT, P, KT, P], BF16, kind="ExternalInput").ap()
    tr = nc.dram_tensor("tr", [P, FT], F32, kind="ExternalInput").ap()
    bt = nc.dram_tensor("bt", [P, FT], F32, kind="ExternalInput").ap()
    ot = nc.dram_tensor("ot", [FT, P, T], F32, kind="ExternalOutput").ap()

    with tile.TileContext(nc) as tc:
        with (
            tc.tile_pool(name="const", bufs=1) as cpool,
            tc.tile_pool(name="wpool", bufs=4) as wpool,
            tc.tile_pool(name="opool", bufs=3) as opool,
            tc.tile_pool(name="psacc", bufs=4, space="PSUM") as pspool,
        ):
            xs_sb = cpool.tile([P, KT, T], BF16)
            tr_sb = cpool.tile([P, FT], F32)
            nc.scalar.dma_start(out=tr_sb, in_=tr)
            bias_sb = cpool.tile([P, FT], F32)
            nc.scalar.dma_start(out=bias_sb, in_=bt)
            for k in range(0, KT, 2):
                nc.sync.dma_start(out=xs_sb[:, k, :], in_=xh[:, k, :])
            for k in range(1, KT, 2):
                nc.scalar.dma_start(out=xs_sb[:, k, :], in_=xh[:, k, :])

            warm = cpool.tile([P, P], F32)
            nc.vector.memset(warm, 0.0)

            def dummy_mms(n, name):
                dps = pspool.tile([P, TCH], F32, tag="accq", bufs=4,
                                  name=name)
                for _ in range(n):
                    nc.tensor.matmul(dps[:, :P // 2],
                                     lhsT=warm.bitcast(F32R),
                                     rhs=warm[:, :P // 2].bitcast(F32R),
                                     start=True, stop=True)

            dummy_mms(14, "warm_ps")

            for ft in range(FT):
                wt_sb = wpool.tile([P, KT, P], BF16, tag="wt")
                nc.gpsimd.dma_start(out=wt_sb, in_=wt[ft])
                out_sb = opool.tile([P, T], F32, tag="out")
                last = ft == FT - 1
                ntc, tch = (4, T // 4) if last else (NTC, TCH)
                for tci in range(ntc):
                    ps = pspool.tile([P, tch], F32,
                                     tag="accq" if last else "acc",
                                     bufs=4)
                    for k in range(KT):
                        nc.tensor.matmul(
                            ps,
                            lhsT=wt_sb[:, k, :],
                            rhs=xs_sb[:, k, tci * tch:(tci + 1) * tch],
                            start=(k == 0), stop=(k == KT - 1),
                        )
                    nc.scalar.activation(
                        out_sb[:, tci * tch:(tci + 1) * tch], ps,
                        mybir.ActivationFunctionType.Identity,
                        bias=bias_sb[:, ft:ft + 1],
                        scale=tr_sb[:, ft:ft + 1],
                    )
                    if last:
                        nc.sync.dma_start(
                            out=ot[ft, :, tci * tch:(tci + 1) * tch],
                            in_=out_sb[:, tci * tch:(tci + 1) * tch])
                if not last:
                    nc.sync.dma_start(out=ot[ft], in_=out_sb)

    nc.compile()
    return nc


def kernel(x, cluster, weight, bias, style_L, style_R):
    global _cached, LAST_RESULT
    x = np.ascontiguousarray(np.asarray(x, dtype=np.float32))
    cluster = np.ascontiguousarray(np.asarray(cluster, dtype=np.float32))
    weight = np.ascontiguousarray(np.asarray(weight, dtype=np.float32))
    bias = np.ascontiguousarray(np.asarray(bias, dtype=np.float32))
    style_L = np.ascontiguousarray(np.asarray(style_L, dtype=np.float32))
    style_R = np.ascontiguousarray(np.asarray(style_R, dtype=np.float32))

    if _cached is None:
        _cached = _build()
    nc = _cached

    tmp_L = cluster @ style_L
    tmp_R = cluster @ style_R
    xs = (x * tmp_L[:, None, :]).astype(ml_dtypes.bfloat16)
    xh_all = np.ascontiguousarray(
        xs.reshape(B, T, KT, P).transpose(0, 3, 2, 1))
    w5 = np.ascontiguousarray(
        weight.astype(ml_dtypes.bfloat16).reshape(FT, P, KT, P)
        .transpose(0, 3, 2, 1))
    tr_all = np.ascontiguousarray(
        tmp_R.reshape(B, FT, P).transpose(0, 2, 1))
    bt = np.ascontiguousarray(bias.reshape(FT, P).T)

    in_maps = [
        {"xh": xh_all[c], "wt": w5, "tr": tr_all[c], "bt": bt}
        for c in range(N_CORES)
    ]

    res = run_bass_kernel_spmd(nc, in_maps, core_ids=list(range(N_CORES)),
                               trace=TRACE)
    LAST_RESULT = res

    out = np.empty((B, T, NF), dtype=np.float32)
    for c in range(N_CORES):
        otc = res.results[c]["ot"]
        out[c] = otc.transpose(2, 0, 1).reshape(T, NF)
    return out

